# revision 28
# baseline (speedup 1.0000x reference)
"""Multi-head attention (B=4, N=2048, E=512, H=8) on 8 Trainium2 cores.

Sharding: core c -> (batch b = c//2, head-group g = c%2 of 4 heads).
Each core computes q/k/v projections for its 4 heads, full attention,
and a partial output projection (its heads' slice of Wo contraction);
the host sums the two partials per batch.

Device data flow (per core, all matmul inputs fp16, accumulation f32):
  - host supplies transposed inputs xqT/xkT/xvT [E, N] and weight slices
    (q/k weights dup-interleaved so each head's projection lands as a
    [128, N] tile with the head's 64 dims duplicated in both partition
    halves -> row-tiled (64-row) energy matmuls run pairwise-concurrent
    at full PE rate)
  - energy (transposed): attT[kc] [128(nk), 2048(nq)] = exp(k_chunk @ qT * s)
    via single K=64 matmuls, exp on ACT straight from PSUM (FD=2048)
  - att @ v_aug: v has a ones column appended, so one accumulated matmul
    chain yields [nq, 64] unnormalized output AND the softmax denominator
  - normalize with per-partition reciprocal (DVE), PE-transpose the
    [nq, 256] result, output projection against WoT slice.
"""

import sys

if "/opt/trn_rl_repo" not in sys.path:
    sys.path.insert(0, "/opt/trn_rl_repo")

import numpy as np

B, N, E, H, D = 4, 2048, 512, 8, 64
NH = 4                      # heads per core
NCHUNK = N // 128           # 16
ECHUNK = E // 128           # 4
SCALE = float(1.0 / np.sqrt(E))
N_CORES = 8

ATT_POOL_BUFS = 30          # shared [128,2048] fp16 slots: 12 xT tiles + 2-head attT window

_built = None


def _build():
    global _built
    if _built is not None:
        return _built

    from contextlib import ExitStack

    import concourse.bass as bass  # noqa: F401
    import concourse.mybir as mybir
    import concourse.tile as tile
    from concourse import bacc

    DT = mybir.dt.float16
    F32 = mybir.dt.float32
    AF = mybir.ActivationFunctionType

    nc = bacc.Bacc(
        "TRN2",
        target_bir_lowering=False,
        debug=False,
        num_devices=N_CORES,
    )

    xqT = nc.dram_tensor("xqT", [E, N], DT, kind="ExternalInput").ap()
    xkT = nc.dram_tensor("xkT", [E, N], DT, kind="ExternalInput").ap()
    xvT = nc.dram_tensor("xvT", [E, N], DT, kind="ExternalInput").ap()
    wqtd = nc.dram_tensor("wqtd", [E, 256], DT, kind="ExternalInput").ap()
    wktd = nc.dram_tensor("wktd", [E, 256], DT, kind="ExternalInput").ap()
    wvt = nc.dram_tensor("wvt", [E, NH * D], DT, kind="ExternalInput").ap()
    wot = nc.dram_tensor("wot", [NH * D, E], DT, kind="ExternalInput").ap()
    iden = nc.dram_tensor("iden", [128, 128], DT, kind="ExternalInput").ap()
    out = nc.dram_tensor("out", [N, E], F32, kind="ExternalOutput").ap()

    with tile.TileContext(nc) as tc, ExitStack() as ctx:
        consts = ctx.enter_context(tc.tile_pool(name="consts", bufs=1))
        big = ctx.enter_context(tc.tile_pool(name="big", bufs=ATT_POOL_BUFS))
        qk = ctx.enter_context(tc.tile_pool(name="qk", bufs=1))
        vp = ctx.enter_context(tc.tile_pool(name="vp", bufs=1))
        oallp = ctx.enter_context(tc.tile_pool(name="oall", bufs=1))
        otp = ctx.enter_context(tc.tile_pool(name="ot", bufs=1))
        ostage = ctx.enter_context(tc.tile_pool(name="ostage", bufs=3))
        stp = ctx.enter_context(tc.tile_pool(name="stp", bufs=1))
        smallp = ctx.enter_context(tc.tile_pool(name="small", bufs=4))

        # PSUM budget (8 banks): energy fp16 [128,2048] = 2 banks x2 bufs,
        # attv [128,65] = 1 bank x2, fin/proj/transpose [128,512]f32 = 1 bank x2
        ps_big = ctx.enter_context(tc.tile_pool(name="ps_big", bufs=3, space="PSUM"))
        ps_av = ctx.enter_context(tc.tile_pool(name="ps_av", bufs=2, space="PSUM"))
        ps_fin = ps_av  # share the same 2 banks (tag-distinct tiles)

        # ---- constant / weight loads ----
        iden_sb = consts.tile([128, 128], DT, tag="iden", name="iden_sb")
        nc.sync.dma_start(out=iden_sb[:], in_=iden[:])
        wq_sb = [consts.tile([128, 256], DT, tag=f"wq{kc}", name=f"wq_sb{kc}") for kc in range(ECHUNK)]
        wk_sb = [consts.tile([128, 256], DT, tag=f"wk{kc}", name=f"wk_sb{kc}") for kc in range(ECHUNK)]
        wv_sb = [consts.tile([128, NH * D], DT, tag=f"wv{kc}", name=f"wv_sb{kc}") for kc in range(ECHUNK)]
        wo_sb = [consts.tile([128, E], DT, tag=f"wo{c}", name=f"wo_sb{c}") for c in range(2)]
        for kc in range(ECHUNK):
            nc.sync.dma_start(out=wq_sb[kc][:], in_=wqtd[128 * kc:128 * (kc + 1), :])
            nc.sync.dma_start(out=wk_sb[kc][:], in_=wktd[128 * kc:128 * (kc + 1), :])
            nc.sync.dma_start(out=wv_sb[kc][:], in_=wvt[128 * kc:128 * (kc + 1), :])
        for c in range(2):
            nc.sync.dma_start(out=wo_sb[c][:], in_=wot[128 * c:128 * (c + 1), :])

        # ---- activation inputs (transposed on host) ----
        xq_sb, xk_sb, xv_sb = [], [], []
        for kc in range(ECHUNK):
            t = big.tile([128, N], DT, tag="big", name="xin")
            nc.sync.dma_start(out=t[:], in_=xqT[128 * kc:128 * (kc + 1), :])
            xq_sb.append(t)
        for kc in range(ECHUNK):
            t = big.tile([128, N], DT, tag="big", name="xin")
            nc.sync.dma_start(out=t[:], in_=xkT[128 * kc:128 * (kc + 1), :])
            xk_sb.append(t)
        for kc in range(ECHUNK):
            t = big.tile([128, N], DT, tag="big", name="xin")
            nc.sync.dma_start(out=t[:], in_=xvT[128 * kc:128 * (kc + 1), :])
            xv_sb.append(t)

        # ---- q/k projections ----
        # qnd[mc]/knd[mc] [128, N]: natural head-pair chunks (head 2mc at
        # rows 0:64, head 2mc+1 at 64:128). qdp/kdp are the swapped copies
        # (made by 2 sbuf->sbuf DMAs) so each head has its 64 dims available
        # in BOTH partition halves -> row-tiled energy matmuls at full rate.
        qnd = [qk.tile([128, N], DT, tag=f"qnd{mc}", name="qnd") for mc in range(2)]
        knd = [qk.tile([128, N], DT, tag=f"knd{mc}", name="knd") for mc in range(2)]
        qdp = [qk.tile([128, N], DT, tag=f"qdp{mc}", name="qdp") for mc in range(2)]
        kdp = [qk.tile([128, N], DT, tag=f"kdp{mc}", name="kdp") for mc in range(2)]

        def emit_proj_qk(mc):
            for (w_sb, x_sb, nd, dp) in (
                (wq_sb, xq_sb, qnd, qdp),
                (wk_sb, xk_sb, knd, kdp),
            ):
                for ns in range(4):
                    ps = ps_fin.tile([128, 512], F32, tag="av", name="ps")
                    for kc in range(ECHUNK):
                        nc.tensor.matmul(
                            ps[:],
                            w_sb[kc][:, 128 * mc:128 * (mc + 1)],
                            x_sb[kc][:, 512 * ns:512 * (ns + 1)],
                            start=(kc == 0),
                            stop=(kc == ECHUNK - 1),
                        )
                    nc.vector.tensor_copy(nd[mc][:, 512 * ns:512 * (ns + 1)], ps[:])
                nc.sync.dma_start(out=dp[mc][0:64, :], in_=nd[mc][64:128, :])
                nc.sync.dma_start(out=dp[mc][64:128, :], in_=nd[mc][0:64, :])

        def half_ap(nd, dp, i, half):
            """[64, N] view of head i's projected data at partition `half`."""
            mc, r = divmod(i, 2)
            if half == 0:
                t = nd[mc] if r == 0 else dp[mc]
                return t[0:64, :]
            t = dp[mc] if r == 0 else nd[mc]
            return t[64:128, :]

        # ---- v projection into augmented layout vsb[kc] [128, NH*65] ----
        # col 65*i + 64 is the ones column for head i (softmax denominator).
        vsb = []

        def emit_vproj():
            for mk in range(NCHUNK):
                ps = ps_fin.tile([128, E], F32, tag="av", name="psf")
                for kc in range(ECHUNK):
                    nc.tensor.matmul(
                        ps[:, 0:NH * D],
                        xv_sb[kc][:, 128 * mk:128 * (mk + 1)],
                        wv_sb[kc][:],
                        start=(kc == 0),
                        stop=(kc == ECHUNK - 1),
                    )
                t = vp.tile([128, NH * 65], DT, tag=f"v{mk}", name=f"v_sb{mk}")
                vsrc = ps[:, 0:NH * D].rearrange("p (h d) -> p h d", h=NH)
                vdst = t[:].rearrange("p (h d) -> p h d", h=NH)[:, :, 0:D]
                nc.vector.tensor_copy(vdst, vsrc)
                ones_cols = t[:].rearrange("p (h d) -> p h d", h=NH)[:, :, D:D + 1]
                nc.vector.memset(ones_cols, 1.0)
                vsb.append(t)

        # ---- attention ----
        oall = [oallp.tile([128, NH * D], DT, tag=f"oall{m}", name=f"oall{m}") for m in range(NCHUNK)]
        ot = [otp.tile([128, N], DT, tag=f"ot{c}", name=f"ot{c}") for c in range(2)]
        st1 = [stp.tile([128, E], DT, tag=f"st1_{m}", name=f"st1_{m}") for m in range(NCHUNK)]

        def emit_energy(i):
            """attT tiles for head i: exp(k_chunk @ q_h.T * SCALE), [128, nq]."""
            tiles = []
            for kc in range(NCHUNK):
                # two [128,1024] f32 psum tiles (2 banks each) per chunk, one
                # per PE row group (partition halves run as concurrent
                # row-tiled matmuls); bufs=2 keeps ACT streaming while PE
                # fills the next chunk
                att = big.tile([128, N], DT, tag="big", name="att")
                for half, ns in ((0, 0), (64, 1)):
                    ps = ps_big.tile([128, N // 2], F32, tag="big", name="ps")
                    kh = half_ap(knd, kdp, i, half)
                    qh = half_ap(qnd, qdp, i, half)
                    for j in range(2):
                        nc.tensor.matmul(
                            ps[:, 512 * j:512 * (j + 1)],
                            kh[:, 128 * kc:128 * (kc + 1)],
                            qh[:, 1024 * ns + 512 * j:1024 * ns + 512 * (j + 1)],
                            start=True,
                            stop=True,
                        )
                    nc.scalar.activation(
                        att[:, 1024 * ns:1024 * (ns + 1)], ps[:], AF.Exp, scale=SCALE
                    )
                tiles.append(att)
            return tiles

        def emit_half_tail(m, c, accum):
            """Transpose head-pair c's slice of oall[m] and apply its Wo rows.
            c=0 (heads 0,1) runs mid-kernel into fp16 st1; c=1 adds heads
            2,3 and emits the final f32 row block."""
            pt = ps_fin.tile([128, 128], DT, tag="av", name="pt")
            nc.tensor.transpose(pt[:], oall[m][:, 128 * c:128 * (c + 1)], iden_sb[:])
            nc.vector.tensor_copy(ot[c][:, 128 * m:128 * (m + 1)], pt[:])
            pf = ps_fin.tile([128, E], F32, tag="av", name="pff")
            nc.tensor.matmul(
                pf[:], ot[c][:, 128 * m:128 * (m + 1)], wo_sb[c][:],
                start=True, stop=True,
            )
            if not accum:
                nc.vector.tensor_copy(st1[m][:], pf[:])
            else:
                st = ostage.tile([128, E], F32, tag="st", name="st")
                nc.vector.tensor_add(st[:], st1[m][:], pf[:])
                nc.sync.dma_start(out=out[128 * m:128 * (m + 1), :], in_=st[:])

        def emit_attv(i, att_tiles, fuse_tail=False):
            for m in range(NCHUNK):
                pav = ps_av.tile([128, 65], F32, tag="av", name="pav")
                for kc in range(NCHUNK):
                    nc.tensor.matmul(
                        pav[:],
                        att_tiles[kc][:, 128 * m:128 * (m + 1)],
                        vsb[kc][:, 65 * i:65 * i + 65],
                        start=(kc == 0),
                        stop=(kc == NCHUNK - 1),
                    )
                rec = smallp.tile([128, 1], F32, tag="rec", name="rec")
                nc.vector.reciprocal(rec[:], pav[:, 64:65])
                nc.vector.tensor_scalar_mul(
                    oall[m][:, D * i:D * (i + 1)], pav[:, 0:D], rec[:]
                )
                if fuse_tail:
                    emit_half_tail(m, 1, True)

        def emit_warm(n_mm=12):
            """Dense N=512 matmuls with a reused stationary operand: ~100%
            PE-busy streak that flips the HAM clock gate to 2.4 GHz."""
            ps = ps_fin.tile([128, 512], F32, tag="av", name="warm")
            for _ in range(n_mm):
                nc.tensor.matmul(ps[:], iden_sb[:], wo_sb[0][:], start=True, stop=True)

        # software-pipelined emission: head i's energy feeds ACT while PE
        # fills gaps with projections and head i-1's att@v
        emit_proj_qk(0)
        att0 = emit_energy(0)
        emit_proj_qk(1)
        emit_vproj()
        emit_warm()
        att1 = emit_energy(1)
        emit_attv(0, att0)
        emit_warm()
        att2 = emit_energy(2)
        emit_attv(1, att1)
        for m in range(NCHUNK):
            emit_half_tail(m, 0, False)
        emit_warm()
        att3 = emit_energy(3)
        emit_attv(2, att2)
        emit_warm()
        emit_attv(3, att3, fuse_tail=True)

    nc.compile()
    _built = nc
    return nc


def _host_prep(query, key, value, Wq, Wk, Wv, Wo, c):
    b, g = c // 2, c % 2
    DT = np.float16
    wqtd = np.empty((E, 256), np.float32)
    wktd = np.empty((E, 256), np.float32)
    wvt = np.empty((E, NH * D), np.float32)
    wot = np.empty((NH * D, E), np.float32)
    for i in range(NH):
        h = NH * g + i
        wqtd[:, D * i:D * (i + 1)] = Wq[D * h:D * (h + 1), :].T
        wktd[:, D * i:D * (i + 1)] = Wk[D * h:D * (h + 1), :].T
        wvt[:, D * i:D * (i + 1)] = Wv[D * h:D * (h + 1), :].T
        wot[D * i:D * (i + 1), :] = Wo[:, D * h:D * (h + 1)].T
    return {
        "xqT": np.ascontiguousarray(query[b].T).astype(DT),
        "xkT": np.ascontiguousarray(key[b].T).astype(DT),
        "xvT": np.ascontiguousarray(value[b].T).astype(DT),
        "wqtd": wqtd.astype(DT),
        "wktd": wktd.astype(DT),
        "wvt": wvt.astype(DT),
        "wot": wot.astype(DT),
        "iden": np.eye(128, dtype=DT),
    }


# test.py can flip these to profile
TRACE = False
TRACE_KWARGS = {}
LAST_RESULTS = None


def kernel(query, key, value, Wq, Wk, Wv, Wo):
    global LAST_RESULTS
    from concourse.bass_utils import run_bass_kernel_spmd

    args = [np.asarray(x, dtype=np.float32) for x in (query, key, value, Wq, Wk, Wv, Wo)]
    nc = _build()
    in_maps = [_host_prep(*args, c) for c in range(N_CORES)]
    res = run_bass_kernel_spmd(
        nc, in_maps, core_ids=list(range(N_CORES)), trace=TRACE, **TRACE_KWARGS
    )
    LAST_RESULTS = res
    outp = np.zeros((B, N, E), np.float32)
    for c in range(N_CORES):
        outp[c // 2] += res.results[c]["out"]
    return outp


# revision 29
# speedup vs baseline: 1.1027x; 1.1027x over previous
"""Multi-head attention (B=4, N=2048, E=512, H=8) on 8 Trainium2 cores.

Sharding: core c -> (batch b = c//2, head-group g = c%2 of 4 heads).
Each core computes q/k/v projections for its 4 heads, full attention,
and a partial output projection (its heads' slice of Wo contraction);
the host sums the two partials per batch.

Device data flow (per core, all matmul inputs fp16, accumulation f32):
  - host supplies transposed inputs xqT/xkT/xvT [E, N] and weight slices
    (q/k weights dup-interleaved so each head's projection lands as a
    [128, N] tile with the head's 64 dims duplicated in both partition
    halves -> row-tiled (64-row) energy matmuls run pairwise-concurrent
    at full PE rate)
  - energy (transposed): attT[kc] [128(nk), 2048(nq)] = exp(k_chunk @ qT * s)
    via single K=64 matmuls, exp on ACT straight from PSUM (FD=2048)
  - att @ v_aug: v has a ones column appended, so one accumulated matmul
    chain yields [nq, 64] unnormalized output AND the softmax denominator
  - normalize with per-partition reciprocal (DVE), PE-transpose the
    [nq, 256] result, output projection against WoT slice.
"""

import sys

if "/opt/trn_rl_repo" not in sys.path:
    sys.path.insert(0, "/opt/trn_rl_repo")

import numpy as np

B, N, E, H, D = 4, 2048, 512, 8, 64
NH = 4                      # heads per core
NCHUNK = N // 128           # 16
ECHUNK = E // 128           # 4
SCALE = float(1.0 / np.sqrt(E))
N_CORES = 8

ATT_POOL_BUFS = 34          # shared [128,2048] fp16 slots: 12 xT tiles + 2-head attT window

_built = None


def _build():
    global _built
    if _built is not None:
        return _built

    from contextlib import ExitStack

    import concourse.bass as bass  # noqa: F401
    import concourse.mybir as mybir
    import concourse.tile as tile
    from concourse import bacc

    DT = mybir.dt.float16
    F32 = mybir.dt.float32
    AF = mybir.ActivationFunctionType

    nc = bacc.Bacc(
        "TRN2",
        target_bir_lowering=False,
        debug=False,
        num_devices=N_CORES,
    )

    xqT = nc.dram_tensor("xqT", [E, N], DT, kind="ExternalInput").ap()
    xkT = nc.dram_tensor("xkT", [E, N], DT, kind="ExternalInput").ap()
    xvT = nc.dram_tensor("xvT", [E, N], DT, kind="ExternalInput").ap()
    wqtd = nc.dram_tensor("wqtd", [E, 256], DT, kind="ExternalInput").ap()
    wktd = nc.dram_tensor("wktd", [E, 256], DT, kind="ExternalInput").ap()
    wvt = nc.dram_tensor("wvt", [E, NH * D], DT, kind="ExternalInput").ap()
    wot = nc.dram_tensor("wot", [NH * D, E], DT, kind="ExternalInput").ap()
    iden = nc.dram_tensor("iden", [128, 128], DT, kind="ExternalInput").ap()
    out = nc.dram_tensor("out", [N, E], F32, kind="ExternalOutput").ap()

    with tile.TileContext(nc) as tc, ExitStack() as ctx:
        consts = ctx.enter_context(tc.tile_pool(name="consts", bufs=1))
        big = ctx.enter_context(tc.tile_pool(name="big", bufs=ATT_POOL_BUFS))
        qk = ctx.enter_context(tc.tile_pool(name="qk", bufs=1))
        vp = ctx.enter_context(tc.tile_pool(name="vp", bufs=1))
        oallp = ctx.enter_context(tc.tile_pool(name="oall", bufs=1))
        otp = ctx.enter_context(tc.tile_pool(name="ot", bufs=1))
        ostage = ctx.enter_context(tc.tile_pool(name="ostage", bufs=3))
        smallp = ctx.enter_context(tc.tile_pool(name="small", bufs=4))

        # PSUM budget (8 banks): energy fp16 [128,2048] = 2 banks x2 bufs,
        # attv [128,65] = 1 bank x2, fin/proj/transpose [128,512]f32 = 1 bank x2
        ps_big = ctx.enter_context(tc.tile_pool(name="ps_big", bufs=3, space="PSUM"))
        ps_av = ctx.enter_context(tc.tile_pool(name="ps_av", bufs=2, space="PSUM"))
        ps_fin = ps_av  # share the same 2 banks (tag-distinct tiles)

        # ---- constant / weight loads ----
        iden_sb = consts.tile([128, 128], DT, tag="iden", name="iden_sb")
        nc.sync.dma_start(out=iden_sb[:], in_=iden[:])
        wq_sb = [consts.tile([128, 256], DT, tag=f"wq{kc}", name=f"wq_sb{kc}") for kc in range(ECHUNK)]
        wk_sb = [consts.tile([128, 256], DT, tag=f"wk{kc}", name=f"wk_sb{kc}") for kc in range(ECHUNK)]
        wv_sb = [consts.tile([128, NH * D], DT, tag=f"wv{kc}", name=f"wv_sb{kc}") for kc in range(ECHUNK)]
        wo_sb = [consts.tile([128, E], DT, tag=f"wo{c}", name=f"wo_sb{c}") for c in range(2)]
        for kc in range(ECHUNK):
            nc.sync.dma_start(out=wq_sb[kc][:], in_=wqtd[128 * kc:128 * (kc + 1), :])
            nc.sync.dma_start(out=wk_sb[kc][:], in_=wktd[128 * kc:128 * (kc + 1), :])
            nc.sync.dma_start(out=wv_sb[kc][:], in_=wvt[128 * kc:128 * (kc + 1), :])
        for c in range(2):
            nc.sync.dma_start(out=wo_sb[c][:], in_=wot[128 * c:128 * (c + 1), :])

        # ---- activation inputs (transposed on host) ----
        xq_sb, xk_sb, xv_sb = [], [], []
        for kc in range(ECHUNK):
            t = big.tile([128, N], DT, tag="big", name="xin")
            nc.sync.dma_start(out=t[:], in_=xqT[128 * kc:128 * (kc + 1), :])
            xq_sb.append(t)
        for kc in range(ECHUNK):
            t = big.tile([128, N], DT, tag="big", name="xin")
            nc.sync.dma_start(out=t[:], in_=xkT[128 * kc:128 * (kc + 1), :])
            xk_sb.append(t)
        for kc in range(ECHUNK):
            t = big.tile([128, N], DT, tag="big", name="xin")
            nc.sync.dma_start(out=t[:], in_=xvT[128 * kc:128 * (kc + 1), :])
            xv_sb.append(t)

        # ---- q/k projections ----
        # qnd[mc]/knd[mc] [128, N]: natural head-pair chunks (head 2mc at
        # rows 0:64, head 2mc+1 at 64:128). qdp/kdp are the swapped copies
        # (made by 2 sbuf->sbuf DMAs) so each head has its 64 dims available
        # in BOTH partition halves -> row-tiled energy matmuls at full rate.
        qnd = [qk.tile([128, N], DT, tag=f"qnd{mc}", name="qnd") for mc in range(2)]
        knd = [qk.tile([128, N], DT, tag=f"knd{mc}", name="knd") for mc in range(2)]
        qdp = [qk.tile([128, N], DT, tag=f"qdp{mc}", name="qdp") for mc in range(2)]
        kdp = [qk.tile([128, N], DT, tag=f"kdp{mc}", name="kdp") for mc in range(2)]

        def emit_proj_qk(mc):
            for (w_sb, x_sb, nd, dp) in (
                (wq_sb, xq_sb, qnd, qdp),
                (wk_sb, xk_sb, knd, kdp),
            ):
                for ns in range(4):
                    ps = ps_fin.tile([128, 512], F32, tag="av", name="ps")
                    for kc in range(ECHUNK):
                        nc.tensor.matmul(
                            ps[:],
                            w_sb[kc][:, 128 * mc:128 * (mc + 1)],
                            x_sb[kc][:, 512 * ns:512 * (ns + 1)],
                            start=(kc == 0),
                            stop=(kc == ECHUNK - 1),
                        )
                    nc.vector.tensor_copy(nd[mc][:, 512 * ns:512 * (ns + 1)], ps[:])
                nc.sync.dma_start(out=dp[mc][0:64, :], in_=nd[mc][64:128, :])
                nc.sync.dma_start(out=dp[mc][64:128, :], in_=nd[mc][0:64, :])

        def half_ap(nd, dp, i, half):
            """[64, N] view of head i's projected data at partition `half`."""
            mc, r = divmod(i, 2)
            if half == 0:
                t = nd[mc] if r == 0 else dp[mc]
                return t[0:64, :]
            t = dp[mc] if r == 0 else nd[mc]
            return t[64:128, :]

        # ---- v projection into augmented layout vsb[kc] [128, NH*65] ----
        # col 65*i + 64 is the ones column for head i (softmax denominator).
        vsb = []

        def emit_vproj():
            for mk in range(NCHUNK):
                ps = ps_fin.tile([128, E], F32, tag="av", name="psf")
                for kc in range(ECHUNK):
                    nc.tensor.matmul(
                        ps[:, 0:NH * D],
                        xv_sb[kc][:, 128 * mk:128 * (mk + 1)],
                        wv_sb[kc][:],
                        start=(kc == 0),
                        stop=(kc == ECHUNK - 1),
                    )
                t = vp.tile([128, NH * 65], DT, tag=f"v{mk}", name=f"v_sb{mk}")
                vsrc = ps[:, 0:NH * D].rearrange("p (h d) -> p h d", h=NH)
                vdst = t[:].rearrange("p (h d) -> p h d", h=NH)[:, :, 0:D]
                nc.vector.tensor_copy(vdst, vsrc)
                ones_cols = t[:].rearrange("p (h d) -> p h d", h=NH)[:, :, D:D + 1]
                nc.vector.memset(ones_cols, 1.0)
                vsb.append(t)

        # ---- attention ----
        oall = [oallp.tile([128, NH * D], DT, tag=f"oall{m}", name=f"oall{m}") for m in range(NCHUNK)]
        ot = [otp.tile([128, N], DT, tag=f"ot{c}", name=f"ot{c}") for c in range(2)]

        def emit_energy(i):
            """attT tiles for head i: exp(k_chunk @ q_h.T * SCALE), [128, nq]."""
            tiles = []
            for kc in range(NCHUNK):
                # two [128,1024] f32 psum tiles (2 banks each) per chunk, one
                # per PE row group (partition halves run as concurrent
                # row-tiled matmuls); bufs=2 keeps ACT streaming while PE
                # fills the next chunk
                att = big.tile([128, N], DT, tag="big", name="att")
                for half, ns in ((0, 0), (64, 1)):
                    ps = ps_big.tile([128, N // 2], F32, tag="big", name="ps")
                    kh = half_ap(knd, kdp, i, half)
                    qh = half_ap(qnd, qdp, i, half)
                    for j in range(2):
                        nc.tensor.matmul(
                            ps[:, 512 * j:512 * (j + 1)],
                            kh[:, 128 * kc:128 * (kc + 1)],
                            qh[:, 1024 * ns + 512 * j:1024 * ns + 512 * (j + 1)],
                            start=True,
                            stop=True,
                        )
                    nc.scalar.activation(
                        att[:, 1024 * ns:1024 * (ns + 1)], ps[:], AF.Exp, scale=SCALE
                    )
                tiles.append(att)
            return tiles

        def emit_tail(m):
            """PE-transpose oall[m] into ot and apply the Wo slice."""
            for c in range(2):
                pt = ps_fin.tile([128, 128], DT, tag="av", name="pt")
                nc.tensor.transpose(pt[:], oall[m][:, 128 * c:128 * (c + 1)], iden_sb[:])
                nc.vector.tensor_copy(ot[c][:, 128 * m:128 * (m + 1)], pt[:])
            pf = ps_fin.tile([128, E], F32, tag="av", name="pff")
            for c in range(2):
                nc.tensor.matmul(
                    pf[:],
                    ot[c][:, 128 * m:128 * (m + 1)],
                    wo_sb[c][:],
                    start=(c == 0),
                    stop=(c == 1),
                )
            st = ostage.tile([128, E], F32, tag="st", name="st")
            nc.vector.tensor_copy(st[:], pf[:])
            nc.sync.dma_start(out=out[128 * m:128 * (m + 1), :], in_=st[:])

        def emit_attv(i, att_tiles, fuse_tail=False):
            for m in range(NCHUNK):
                pav = ps_av.tile([128, 65], F32, tag="av", name="pav")
                for kc in range(NCHUNK):
                    nc.tensor.matmul(
                        pav[:],
                        att_tiles[kc][:, 128 * m:128 * (m + 1)],
                        vsb[kc][:, 65 * i:65 * i + 65],
                        start=(kc == 0),
                        stop=(kc == NCHUNK - 1),
                    )
                rec = smallp.tile([128, 1], F32, tag="rec", name="rec")
                nc.vector.reciprocal(rec[:], pav[:, 64:65])
                nc.vector.tensor_scalar_mul(
                    oall[m][:, D * i:D * (i + 1)], pav[:, 0:D], rec[:]
                )
                if fuse_tail:
                    emit_tail(m)

        def emit_warm(n_mm=12):
            """Dense N=512 matmuls with a reused stationary operand: ~100%
            PE-busy streak that flips the HAM clock gate to 2.4 GHz."""
            ps = ps_fin.tile([128, 512], F32, tag="av", name="warm")
            for _ in range(n_mm):
                nc.tensor.matmul(ps[:], iden_sb[:], wo_sb[0][:], start=True, stop=True)

        # software-pipelined emission: head i's energy feeds ACT while PE
        # fills gaps with projections and head i-1's att@v
        emit_proj_qk(0)
        att0 = emit_energy(0)
        emit_proj_qk(1)
        emit_vproj()
        emit_warm()
        att1 = emit_energy(1)
        emit_attv(0, att0)
        emit_warm()
        att2 = emit_energy(2)
        emit_attv(1, att1)
        emit_warm()
        att3 = emit_energy(3)
        emit_attv(2, att2)
        emit_warm()
        emit_attv(3, att3, fuse_tail=True)

    nc.compile()
    _built = nc
    return nc


def _host_prep(query, key, value, Wq, Wk, Wv, Wo, c):
    b, g = c // 2, c % 2
    DT = np.float16
    wqtd = np.empty((E, 256), np.float32)
    wktd = np.empty((E, 256), np.float32)
    wvt = np.empty((E, NH * D), np.float32)
    wot = np.empty((NH * D, E), np.float32)
    for i in range(NH):
        h = NH * g + i
        wqtd[:, D * i:D * (i + 1)] = Wq[D * h:D * (h + 1), :].T
        wktd[:, D * i:D * (i + 1)] = Wk[D * h:D * (h + 1), :].T
        wvt[:, D * i:D * (i + 1)] = Wv[D * h:D * (h + 1), :].T
        wot[D * i:D * (i + 1), :] = Wo[:, D * h:D * (h + 1)].T
    return {
        "xqT": np.ascontiguousarray(query[b].T).astype(DT),
        "xkT": np.ascontiguousarray(key[b].T).astype(DT),
        "xvT": np.ascontiguousarray(value[b].T).astype(DT),
        "wqtd": wqtd.astype(DT),
        "wktd": wktd.astype(DT),
        "wvt": wvt.astype(DT),
        "wot": wot.astype(DT),
        "iden": np.eye(128, dtype=DT),
    }


# test.py can flip these to profile
TRACE = False
TRACE_KWARGS = {}
LAST_RESULTS = None


def kernel(query, key, value, Wq, Wk, Wv, Wo):
    global LAST_RESULTS
    from concourse.bass_utils import run_bass_kernel_spmd

    args = [np.asarray(x, dtype=np.float32) for x in (query, key, value, Wq, Wk, Wv, Wo)]
    nc = _build()
    in_maps = [_host_prep(*args, c) for c in range(N_CORES)]
    res = run_bass_kernel_spmd(
        nc, in_maps, core_ids=list(range(N_CORES)), trace=TRACE, **TRACE_KWARGS
    )
    LAST_RESULTS = res
    outp = np.zeros((B, N, E), np.float32)
    for c in range(N_CORES):
        outp[c // 2] += res.results[c]["out"]
    return outp
